# revision 1
# baseline (speedup 1.0000x reference)
"""Trainium2 Bass kernel for nn_Net_28544352649361 (segment_reduce).

Reference computation:
    emb_tok[t]   = sum_d word_vectors[tokens[t], d]
    seg_sum[s]   = segment_sum(emb_tok, segment_ids)    (segment_ids sorted)
    lengths[s]   = segment counts
    sv[s]        = seg_sum[s] / max(lengths[s], 1)
    out[s, l]    = sv[s] * sum_d hidden_w[l, d] + hidden_b[l]
(the reference broadcasts the per-sentence scalar over d, so the final Linear
collapses to an outer product against hidden_w's row sums).

Distribution: data-parallel over sentences. Host cuts the token stream at
sentence boundaries s = 2048*c (8 binary searches), pads each shard to a fixed
135168 tokens, and runs one SPMD Bass program on all 8 NeuronCores. Each core:
  - per 128-token column: indirect-DMA row gather wv[tok], DVE reduce over d,
    indirect scatter-ADD (DMA CCE) of [emb, 1.0] into a DRAM accumulator
    indexed by in-shard segment id. 128 rotating accumulators keep concurrent
    scatter-adds race-free (columns 128 apart never share a segment).
  - combine accumulators, sv = sum/max(cnt,1), outer product with the
    broadcast row-sums of hidden_w, add bias, write [2048, 128] rows.
Host concatenates the 8 row blocks.
"""

import sys

sys.path.insert(0, "/opt/trn_rl_repo")

from contextlib import ExitStack

import numpy as np

import concourse.bass as bass
import concourse.tile as tile
from concourse import mybir
from concourse.bass import IndirectOffsetOnAxis
from concourse.vector_clock import ScopedClock

P = 128
F = 1056                 # token columns per core (128*1056 = 135168 slots)
SHARD = P * F
D = 128
NL = 128
NSENT = 16384
NCORES = 8
SENT_PER_CORE = NSENT // NCORES   # 2048
NBAG = 128               # rotating scatter-add accumulators
BAGROWS = 16640          # >= SENT_PER_CORE, pad slot at PAD_SEG
PAD_SEG = 16500          # in-shard segment id used for padding tokens
UROWS = SENT_PER_CORE // P        # 16

_num_splits = [0]


# ---------------------------------------------------------------------------
# Workarounds for this walrus build (accepts only ONE sync-wait per
# instruction) and Tile's 8-lane DMA-sem round robin.
# ---------------------------------------------------------------------------
def _split_drain_and_barrier(self, tick_clock, wait_clock):
    nc = self.nc
    drain_inst = nc.sync.drain()
    wait_clock.add_sem_waits(
        drain_inst.ins, ScopedClock({None: tick_clock.global_clock})
    )
    mi = drain_inst.ins
    si = mi.sync_info
    if si is not None and si.on_wait is not None and len(si.on_wait) > 1:
        waits = list(si.on_wait)
        si.on_wait = waits[:1]
        for w in waits[1:]:
            extra = nc.sync.drain()
            emi = extra.ins
            esi = emi.sync_info
            if esi is None:
                emi.sync_info = mybir.SyncInfo(on_wait=[w], on_update=[])
            else:
                esi.on_wait = [w]
    nc.all_engine_barrier()
    assert self.sems is not None
    popped = nc._tile_sem_poison_stack.pop()
    assert popped is self._sem_poison
    nc.clear_and_free_semaphores(list(self.sems.allocated().values()))
    nc.all_engine_barrier()


def _apply_patches():
    if getattr(tile, "_segred_patched", False):
        return
    tile.TileContext._drain_and_barrier = _split_drain_and_barrier
    import concourse.tile_sem_assignment as tsa

    tsa.NUM_SWDGE_GLOBAL_SEMS = 1
    tsa.NUM_HWDGE_SEMS = 1
    tile._segred_patched = True


def _split_waits(nc):
    """Hoist surplus sync-waits onto same-engine NoOps placed just before the
    waiter; the engine sequencer executes them in order."""
    import bass_rust

    for f in nc.m.functions:
        for bb in f.blocks:
            new_list = []
            changed = False
            for inst in bb.instructions:
                si = inst.sync_info
                if si is not None and si.on_wait is not None and len(si.on_wait) > 1:
                    waits = list(si.on_wait)
                    si.on_wait = waits[-1:]
                    for w in waits[:-1]:
                        _num_splits[0] += 1
                        nop = bass_rust.InstNoOp(
                            name=f"WSPLIT-{_num_splits[0]}", ins=[], outs=[]
                        )
                        nop.engine = inst.engine
                        nop.sync_info = mybir.SyncInfo(on_wait=[w], on_update=[])
                        new_list.append(nop)
                    changed = True
                new_list.append(inst)
            if changed:
                bb.instructions = new_list


# ---------------------------------------------------------------------------
# Device program (identical for all cores; per-core data via in_maps)
# ---------------------------------------------------------------------------
def build_program():
    _apply_patches()
    nc = bass.Bass()
    f32 = mybir.dt.float32
    i32 = mybir.dt.int32

    wv = nc.declare_dram_parameter("wv", [100352, D], f32, isOutput=False)
    toks = nc.declare_dram_parameter("toks", [P, F], i32, isOutput=False)
    segs = nc.declare_dram_parameter("segs", [P, F], i32, isOutput=False)
    hwT = nc.declare_dram_parameter("hwT", [D, NL], f32, isOutput=False)
    hb = nc.declare_dram_parameter("hb", [1, NL], f32, isOutput=False)
    out = nc.declare_dram_parameter("out", [SENT_PER_CORE, NL], f32, isOutput=True)

    bags = [nc.dram_tensor(f"bag{r}", [BAGROWS, 2], f32) for r in range(NBAG)]

    with ExitStack() as ctx:
        tc = ctx.enter_context(tile.TileContext(nc))
        const = ctx.enter_context(tc.tile_pool(name="const", bufs=1))
        gp = ctx.enter_context(tc.tile_pool(name="g", bufs=6))
        small = ctx.enter_context(tc.tile_pool(name="small", bufs=2))
        psum = ctx.enter_context(tc.tile_pool(name="ps", bufs=2, space="PSUM"))

        tok_sb = const.tile([P, F], i32)
        seg_sb = const.tile([P, F], i32)
        nc.sync.dma_start(tok_sb[:], toks[:])
        nc.sync.dma_start(seg_sb[:], segs[:])

        # zero-init the accumulators
        z = const.tile([P, 2 * BAGROWS // P], f32)
        nc.vector.memset(z[:], 0.0)
        for r in range(NBAG):
            nc.sync.dma_start(bags[r][:], z[:])

        # payload array: [:, k, 0] = emb column k (written later), [:, k, 1] = 1.0
        s2 = const.tile([P, F, 2], f32)
        nc.vector.memset(s2[:], 1.0)

        # main loop: gather rows -> reduce -> scatter-add [emb, 1] into bag
        for k in range(F):
            g = gp.tile([P, D], f32, tag="g")
            nc.gpsimd.indirect_dma_start(
                out=g[:],
                out_offset=None,
                in_=wv[:],
                in_offset=IndirectOffsetOnAxis(ap=tok_sb[:, k : k + 1], axis=0),
            )
            nc.vector.tensor_reduce(
                s2[:, k, 0:1],
                g[:],
                axis=mybir.AxisListType.X,
                op=mybir.AluOpType.add,
            )
            nc.gpsimd.indirect_dma_start(
                out=bags[k % NBAG][:],
                out_offset=IndirectOffsetOnAxis(ap=seg_sb[:, k : k + 1], axis=0),
                in_=s2[:, k, :],
                in_offset=None,
                compute_op=mybir.AluOpType.add,
            )

        # combine the 128 accumulators: acc[p, u, c] = sum_r bag_r[u*128+p, c]
        acc = const.tile([P, UROWS, 2], f32)
        nc.vector.memset(acc[:], 0.0)
        for r in range(NBAG):
            w = gp.tile([P, UROWS, 2], f32, tag="w")
            nc.sync.dma_start(
                w[:],
                bags[r][:].rearrange("(u p) c -> p u c", p=P)[0:P, 0:UROWS, :],
            )
            nc.vector.tensor_tensor(
                out=acc[:], in0=acc[:], in1=w[:], op=mybir.AluOpType.add
            )

        # sv = seg_sum / max(len, 1)
        lens = small.tile([P, UROWS], f32)
        nc.vector.tensor_scalar_max(lens[:], acc[:, :, 1], 1.0)
        rec = small.tile([P, UROWS], f32)
        nc.vector.reciprocal(rec[:], lens[:])
        sv = small.tile([P, UROWS], f32)
        nc.vector.tensor_tensor(
            out=sv[:], in0=acc[:, :, 0], in1=rec[:], op=mybir.AluOpType.mult
        )

        # broadcast row-sums of hidden_w and the bias to all partitions
        hwT_sb = const.tile([D, NL], f32)
        nc.sync.dma_start(hwT_sb[:], hwT[:])
        hb_sb = const.tile([1, NL], f32)
        nc.sync.dma_start(hb_sb[:], hb[:])
        ones_p = const.tile([P, 1], f32)
        nc.vector.memset(ones_p[:], 1.0)
        ones_1 = const.tile([1, P], f32)
        nc.vector.memset(ones_1[:], 1.0)

        wrow_ps = psum.tile([1, NL], f32, tag="ps1")
        nc.tensor.matmul(wrow_ps[:], ones_p[:], hwT_sb[:], start=True, stop=True)
        wrow = small.tile([1, NL], f32)
        nc.scalar.copy(wrow[:], wrow_ps[:])

        wb_ps = psum.tile([P, NL], f32, tag="ps2")
        nc.tensor.matmul(wb_ps[:], ones_1[:], wrow[:], start=True, stop=True)
        w_b = const.tile([P, NL], f32)
        nc.scalar.copy(w_b[:], wb_ps[:])

        bb_ps = psum.tile([P, NL], f32, tag="ps2")
        nc.tensor.matmul(bb_ps[:], ones_1[:], hb_sb[:], start=True, stop=True)
        b_b = const.tile([P, NL], f32)
        nc.scalar.copy(b_b[:], bb_ps[:])

        # out[u*128+p, l] = sv[p, u] * w_b[p, l] + b_b[p, l]
        out_sb = const.tile([P, UROWS, NL], f32)
        for u in range(UROWS):
            nc.vector.tensor_scalar(
                out=out_sb[:, u, :],
                in0=w_b[:],
                scalar1=sv[:, u : u + 1],
                scalar2=None,
                op0=mybir.AluOpType.mult,
            )
            nc.vector.tensor_tensor(
                out=out_sb[:, u, :],
                in0=out_sb[:, u, :],
                in1=b_b[:],
                op=mybir.AluOpType.add,
            )

        nc.sync.dma_start(
            out[:].rearrange("(u p) l -> p u l", p=P)[0:P, 0:UROWS, :], out_sb[:]
        )

    _split_waits(nc)
    return nc


_PROGRAM = None


def _get_program():
    global _PROGRAM
    if _PROGRAM is None:
        _PROGRAM = build_program()
    return _PROGRAM


def kernel(tokens, segment_ids, word_vectors, hidden_w, hidden_b):
    from concourse.bass_utils import run_bass_kernel_spmd

    tokens = np.asarray(tokens)
    segment_ids = np.asarray(segment_ids)
    word_vectors = np.asarray(word_vectors, dtype=np.float32)
    hidden_w = np.asarray(hidden_w, dtype=np.float32)
    hidden_b = np.asarray(hidden_b, dtype=np.float32)

    # replicate-pad the embedding table to the declared 100352 rows
    wv_pad = np.zeros((100352, D), dtype=np.float32)
    wv_pad[: word_vectors.shape[0]] = word_vectors
    hwT = np.ascontiguousarray(hidden_w.T)
    hb = hidden_b.reshape(1, NL)

    # sentence-aligned cuts: core c owns sentences [2048c, 2048(c+1))
    cuts = np.searchsorted(segment_ids, np.arange(NCORES + 1) * SENT_PER_CORE)
    in_maps = []
    for c in range(NCORES):
        lo, hi = int(cuts[c]), int(cuts[c + 1])
        n = hi - lo
        assert n <= SHARD, f"shard {c} has {n} tokens > {SHARD}"
        tk = np.zeros(SHARD, dtype=np.int32)
        sg = np.full(SHARD, PAD_SEG, dtype=np.int32)
        tk[:n] = tokens[lo:hi]
        sg[:n] = segment_ids[lo:hi] - c * SENT_PER_CORE
        in_maps.append(
            {
                "wv": wv_pad,
                "toks": tk.reshape(P, F),
                "segs": sg.reshape(P, F),
                "hwT": hwT,
                "hb": hb,
            }
        )

    nc = _get_program()
    res = run_bass_kernel_spmd(nc, in_maps, list(range(NCORES)))
    return np.concatenate([res.results[c]["out"] for c in range(NCORES)], axis=0)


# revision 2
# speedup vs baseline: 380.6674x; 380.6674x over previous
"""Trainium2 Bass kernel for nn_Net_28544352649361 (segment_reduce).

Reference computation:
    emb_tok[t]   = sum_d word_vectors[tokens[t], d]
    seg_sum[s]   = segment_sum(emb_tok, segment_ids)    (segment_ids sorted)
    lengths[s]   = segment counts
    sv[s]        = seg_sum[s] / max(lengths[s], 1)
    out[s, l]    = sv[s] * sum_d hidden_w[l, d] + hidden_b[l]
(the reference broadcasts the per-sentence scalar over d, so the final Linear
collapses to an outer product against hidden_w's row sums).

Distribution: data-parallel over sentences. Host cuts the token stream at
sentence boundaries s = 2048*c (8 binary searches), pads each shard to a fixed
135168 tokens, and runs one SPMD Bass program on all 8 NeuronCores. Each core:
  - per 128-token column: indirect-DMA row gather wv[tok], DVE reduce over d,
    indirect scatter-ADD (DMA CCE) of [emb, 1.0] into a DRAM accumulator
    indexed by in-shard segment id. 128 rotating accumulators keep concurrent
    scatter-adds race-free (columns 128 apart never share a segment).
  - combine accumulators, sv = sum/max(cnt,1), outer product with the
    broadcast row-sums of hidden_w, add bias, write [2048, 128] rows.
Host concatenates the 8 row blocks.
"""

import sys

sys.path.insert(0, "/opt/trn_rl_repo")

from contextlib import ExitStack

import numpy as np

import concourse.bass as bass
import concourse.tile as tile
from concourse import mybir
from concourse.bass import IndirectOffsetOnAxis
from concourse.vector_clock import ScopedClock

P = 128
F = 1056                 # token columns per core (128*1056 = 135168 slots)
SHARD = P * F
D = 128
NL = 128
NSENT = 16384
NCORES = 8
SENT_PER_CORE = NSENT // NCORES   # 2048
NBAG = 128               # rotating scatter-add accumulators
BAGROWS = 16640          # >= SENT_PER_CORE, pad slot at PAD_SEG
PAD_SEG = 16500          # in-shard segment id used for padding tokens
UROWS = SENT_PER_CORE // P        # 16

_num_splits = [0]


# ---------------------------------------------------------------------------
# Workarounds for this walrus build (accepts only ONE sync-wait per
# instruction) and Tile's 8-lane DMA-sem round robin.
# ---------------------------------------------------------------------------
def _split_drain_and_barrier(self, tick_clock, wait_clock):
    nc = self.nc
    drain_inst = nc.sync.drain()
    wait_clock.add_sem_waits(
        drain_inst.ins, ScopedClock({None: tick_clock.global_clock})
    )
    mi = drain_inst.ins
    si = mi.sync_info
    if si is not None and si.on_wait is not None and len(si.on_wait) > 1:
        waits = list(si.on_wait)
        si.on_wait = waits[:1]
        for w in waits[1:]:
            extra = nc.sync.drain()
            emi = extra.ins
            esi = emi.sync_info
            if esi is None:
                emi.sync_info = mybir.SyncInfo(on_wait=[w], on_update=[])
            else:
                esi.on_wait = [w]
    nc.all_engine_barrier()
    assert self.sems is not None
    popped = nc._tile_sem_poison_stack.pop()
    assert popped is self._sem_poison
    nc.clear_and_free_semaphores(list(self.sems.allocated().values()))
    nc.all_engine_barrier()


def _apply_patches():
    if getattr(tile, "_segred_patched", False):
        return
    tile.TileContext._drain_and_barrier = _split_drain_and_barrier
    import concourse.tile_sem_assignment as tsa

    tsa.NUM_SWDGE_GLOBAL_SEMS = 1
    tsa.NUM_HWDGE_SEMS = 1
    tile._segred_patched = True


def _split_waits(nc):
    """Hoist surplus sync-waits onto same-engine NoOps placed just before the
    waiter; the engine sequencer executes them in order."""
    import bass_rust

    for f in nc.m.functions:
        for bb in f.blocks:
            new_list = []
            changed = False
            for inst in bb.instructions:
                si = inst.sync_info
                if si is not None and si.on_wait is not None and len(si.on_wait) > 1:
                    waits = list(si.on_wait)
                    si.on_wait = waits[-1:]
                    for w in waits[:-1]:
                        _num_splits[0] += 1
                        nop = bass_rust.InstNoOp(
                            name=f"WSPLIT-{_num_splits[0]}", ins=[], outs=[]
                        )
                        nop.engine = inst.engine
                        nop.sync_info = mybir.SyncInfo(on_wait=[w], on_update=[])
                        new_list.append(nop)
                    changed = True
                new_list.append(inst)
            if changed:
                bb.instructions = new_list


# ---------------------------------------------------------------------------
# Device program (identical for all cores; per-core data via in_maps)
# ---------------------------------------------------------------------------
def build_program():
    _apply_patches()
    nc = bass.Bass()
    f32 = mybir.dt.float32
    i32 = mybir.dt.int32

    wv = nc.declare_dram_parameter("wv", [100352, D], f32, isOutput=False)
    toks = nc.declare_dram_parameter("toks", [P, F], i32, isOutput=False)
    segs = nc.declare_dram_parameter("segs", [P, F], i32, isOutput=False)
    hwT = nc.declare_dram_parameter("hwT", [D, NL], f32, isOutput=False)
    hb = nc.declare_dram_parameter("hb", [1, NL], f32, isOutput=False)
    out = nc.declare_dram_parameter("out", [SENT_PER_CORE, NL], f32, isOutput=True)

    bags = [nc.dram_tensor(f"bag{r}", [BAGROWS, 2], f32) for r in range(NBAG)]

    with ExitStack() as ctx:
        tc = ctx.enter_context(tile.TileContext(nc))
        const = ctx.enter_context(tc.tile_pool(name="const", bufs=1))
        gp = ctx.enter_context(tc.tile_pool(name="g", bufs=6))
        small = ctx.enter_context(tc.tile_pool(name="small", bufs=2))
        psum = ctx.enter_context(tc.tile_pool(name="ps", bufs=2, space="PSUM"))

        tok_sb = const.tile([P, F], i32)
        seg_sb = const.tile([P, F], i32)
        nc.sync.dma_start(tok_sb[:], toks[:])
        nc.sync.dma_start(seg_sb[:], segs[:])

        # zero-init the accumulators
        z = const.tile([P, 2 * BAGROWS // P], f32)
        nc.vector.memset(z[:], 0.0)
        for r in range(NBAG):
            nc.sync.dma_start(bags[r][:], z[:])

        # main loop: gather rows -> reduce -> scatter-add [emb, 1] into bag.
        # Payload tiles rotate through a pool so the per-column chains pipeline
        # (a single shared payload array would serialize every chain through
        # tile-granular dependency tracking).
        sp = ctx.enter_context(tc.tile_pool(name="s2", bufs=12))
        for k in range(F):
            g = gp.tile([P, D], f32, tag="g")
            nc.gpsimd.indirect_dma_start(
                out=g[:],
                out_offset=None,
                in_=wv[:],
                in_offset=IndirectOffsetOnAxis(ap=tok_sb[:, k : k + 1], axis=0),
            )
            s2 = sp.tile([P, 2], f32, tag="s2")
            nc.scalar.memzero(s2[:, 1:2])
            nc.scalar.add(s2[:, 1:2], s2[:, 1:2], 1.0)
            nc.vector.tensor_reduce(
                s2[:, 0:1],
                g[:],
                axis=mybir.AxisListType.X,
                op=mybir.AluOpType.add,
            )
            nc.gpsimd.indirect_dma_start(
                out=bags[k % NBAG][:],
                out_offset=IndirectOffsetOnAxis(ap=seg_sb[:, k : k + 1], axis=0),
                in_=s2[:],
                in_offset=None,
                compute_op=mybir.AluOpType.add,
            )

        # combine the 128 accumulators: acc[p, u, c] = sum_r bag_r[u*128+p, c]
        acc = const.tile([P, UROWS, 2], f32)
        nc.vector.memset(acc[:], 0.0)
        for r in range(NBAG):
            w = gp.tile([P, UROWS, 2], f32, tag="w")
            nc.sync.dma_start(
                w[:],
                bags[r][:].rearrange("(u p) c -> p u c", p=P)[0:P, 0:UROWS, :],
            )
            nc.vector.tensor_tensor(
                out=acc[:], in0=acc[:], in1=w[:], op=mybir.AluOpType.add
            )

        # sv = seg_sum / max(len, 1)
        lens = small.tile([P, UROWS], f32)
        nc.vector.tensor_scalar_max(lens[:], acc[:, :, 1], 1.0)
        rec = small.tile([P, UROWS], f32)
        nc.vector.reciprocal(rec[:], lens[:])
        sv = small.tile([P, UROWS], f32)
        nc.vector.tensor_tensor(
            out=sv[:], in0=acc[:, :, 0], in1=rec[:], op=mybir.AluOpType.mult
        )

        # broadcast row-sums of hidden_w and the bias to all partitions
        hwT_sb = const.tile([D, NL], f32)
        nc.sync.dma_start(hwT_sb[:], hwT[:])
        hb_sb = const.tile([1, NL], f32)
        nc.sync.dma_start(hb_sb[:], hb[:])
        ones_p = const.tile([P, 1], f32)
        nc.vector.memset(ones_p[:], 1.0)
        ones_1 = const.tile([1, P], f32)
        nc.vector.memset(ones_1[:], 1.0)

        wrow_ps = psum.tile([1, NL], f32, tag="ps1")
        nc.tensor.matmul(wrow_ps[:], ones_p[:], hwT_sb[:], start=True, stop=True)
        wrow = small.tile([1, NL], f32)
        nc.scalar.copy(wrow[:], wrow_ps[:])

        wb_ps = psum.tile([P, NL], f32, tag="ps2")
        nc.tensor.matmul(wb_ps[:], ones_1[:], wrow[:], start=True, stop=True)
        w_b = const.tile([P, NL], f32)
        nc.scalar.copy(w_b[:], wb_ps[:])

        bb_ps = psum.tile([P, NL], f32, tag="ps2")
        nc.tensor.matmul(bb_ps[:], ones_1[:], hb_sb[:], start=True, stop=True)
        b_b = const.tile([P, NL], f32)
        nc.scalar.copy(b_b[:], bb_ps[:])

        # out[u*128+p, l] = sv[p, u] * w_b[p, l] + b_b[p, l]
        out_sb = const.tile([P, UROWS, NL], f32)
        for u in range(UROWS):
            nc.vector.tensor_scalar(
                out=out_sb[:, u, :],
                in0=w_b[:],
                scalar1=sv[:, u : u + 1],
                scalar2=None,
                op0=mybir.AluOpType.mult,
            )
            nc.vector.tensor_tensor(
                out=out_sb[:, u, :],
                in0=out_sb[:, u, :],
                in1=b_b[:],
                op=mybir.AluOpType.add,
            )

        nc.sync.dma_start(
            out[:].rearrange("(u p) l -> p u l", p=P)[0:P, 0:UROWS, :], out_sb[:]
        )

    _split_waits(nc)
    return nc


_PROGRAM = None


def _get_program():
    global _PROGRAM
    if _PROGRAM is None:
        _PROGRAM = build_program()
    return _PROGRAM


def kernel(tokens, segment_ids, word_vectors, hidden_w, hidden_b):
    from concourse.bass_utils import run_bass_kernel_spmd

    tokens = np.asarray(tokens)
    segment_ids = np.asarray(segment_ids)
    word_vectors = np.asarray(word_vectors, dtype=np.float32)
    hidden_w = np.asarray(hidden_w, dtype=np.float32)
    hidden_b = np.asarray(hidden_b, dtype=np.float32)

    # replicate-pad the embedding table to the declared 100352 rows
    wv_pad = np.zeros((100352, D), dtype=np.float32)
    wv_pad[: word_vectors.shape[0]] = word_vectors
    hwT = np.ascontiguousarray(hidden_w.T)
    hb = hidden_b.reshape(1, NL)

    # sentence-aligned cuts: core c owns sentences [2048c, 2048(c+1))
    cuts = np.searchsorted(segment_ids, np.arange(NCORES + 1) * SENT_PER_CORE)
    in_maps = []
    for c in range(NCORES):
        lo, hi = int(cuts[c]), int(cuts[c + 1])
        n = hi - lo
        assert n <= SHARD, f"shard {c} has {n} tokens > {SHARD}"
        tk = np.zeros(SHARD, dtype=np.int32)
        sg = np.full(SHARD, PAD_SEG, dtype=np.int32)
        tk[:n] = tokens[lo:hi]
        sg[:n] = segment_ids[lo:hi] - c * SENT_PER_CORE
        in_maps.append(
            {
                "wv": wv_pad,
                "toks": tk.reshape(P, F),
                "segs": sg.reshape(P, F),
                "hwT": hwT,
                "hb": hb,
            }
        )

    nc = _get_program()
    res = run_bass_kernel_spmd(nc, in_maps, list(range(NCORES)))
    return np.concatenate([res.results[c]["out"] for c in range(NCORES)], axis=0)


# revision 4
# speedup vs baseline: 2466.9246x; 6.4805x over previous
"""Trainium2 Bass kernel for nn_Net_28544352649361 (segment_reduce).

Reference computation:
    emb_tok[t]   = sum_d word_vectors[tokens[t], d]
    seg_sum[s]   = segment_sum(emb_tok, segment_ids)    (segment_ids sorted)
    lengths[s]   = segment counts
    sv[s]        = seg_sum[s] / max(lengths[s], 1)
    out[s, l]    = sv[s] * sum_d hidden_w[l, d] + hidden_b[l]
(the reference broadcasts the per-sentence scalar over d, so the final Linear
collapses to an outer product against hidden_w's row sums).

Distribution: data-parallel over sentences. Host cuts the token stream at
sentence boundaries s = 2048*c (8 binary searches), pads each shard to a fixed
135168 tokens, and runs one SPMD Bass program on all 8 NeuronCores. Each core:
  - per 128-token column: indirect-DMA row gather wv[tok], DVE reduce over d,
    indirect scatter-ADD (DMA CCE) of [emb, 1.0] into a DRAM accumulator
    indexed by in-shard segment id. 128 rotating accumulators keep concurrent
    scatter-adds race-free (columns 128 apart never share a segment).
  - combine accumulators, sv = sum/max(cnt,1), outer product with the
    broadcast row-sums of hidden_w, add bias, write [2048, 128] rows.
Host concatenates the 8 row blocks.
"""

import sys

sys.path.insert(0, "/opt/trn_rl_repo")

from contextlib import ExitStack

import numpy as np

import concourse.bass as bass
import concourse.tile as tile
from concourse import mybir
from concourse.bass import IndirectOffsetOnAxis
from concourse.vector_clock import ScopedClock

P = 128
F = 1056                 # token columns per core (128*1056 = 135168 slots)
SHARD = P * F
D = 128
NL = 128
NSENT = 16384
NCORES = 8
SENT_PER_CORE = NSENT // NCORES   # 2048
NBAG = 128               # rotating scatter-add accumulators
BAGROWS = 2176           # SENT_PER_CORE + junk rows for padding tokens
PAD_SEG = 2048           # in-shard segment id used for padding tokens (junk row)
UROWS = SENT_PER_CORE // P        # 16

_num_splits = [0]


# ---------------------------------------------------------------------------
# Workarounds for this walrus build (accepts only ONE sync-wait per
# instruction) and Tile's 8-lane DMA-sem round robin.
# ---------------------------------------------------------------------------
def _split_drain_and_barrier(self, tick_clock, wait_clock):
    nc = self.nc
    drain_inst = nc.sync.drain()
    wait_clock.add_sem_waits(
        drain_inst.ins, ScopedClock({None: tick_clock.global_clock})
    )
    mi = drain_inst.ins
    si = mi.sync_info
    if si is not None and si.on_wait is not None and len(si.on_wait) > 1:
        waits = list(si.on_wait)
        si.on_wait = waits[:1]
        for w in waits[1:]:
            extra = nc.sync.drain()
            emi = extra.ins
            esi = emi.sync_info
            if esi is None:
                emi.sync_info = mybir.SyncInfo(on_wait=[w], on_update=[])
            else:
                esi.on_wait = [w]
    nc.all_engine_barrier()
    assert self.sems is not None
    popped = nc._tile_sem_poison_stack.pop()
    assert popped is self._sem_poison
    nc.clear_and_free_semaphores(list(self.sems.allocated().values()))
    nc.all_engine_barrier()


def _apply_patches():
    if getattr(tile, "_segred_patched", False):
        return
    tile.TileContext._drain_and_barrier = _split_drain_and_barrier
    # NOTE: keep all 8 DMA sem lanes — _split_waits() below enforces the
    # 1-sync-wait-per-instruction compiler limit by hoisting surplus waits
    # onto NoOps, and a single lane would make every DMA consumer
    # transitively wait on all earlier DMAs' completions (full round-trip
    # serialization, ~9 us per column).
    tile._segred_patched = True


def _split_waits(nc):
    """Hoist surplus sync-waits onto same-engine NoOps placed just before the
    waiter; the engine sequencer executes them in order."""
    import bass_rust

    for f in nc.m.functions:
        for bb in f.blocks:
            new_list = []
            changed = False
            for inst in bb.instructions:
                si = inst.sync_info
                if si is not None and si.on_wait is not None and len(si.on_wait) > 1:
                    waits = list(si.on_wait)
                    si.on_wait = waits[-1:]
                    for w in waits[:-1]:
                        _num_splits[0] += 1
                        nop = bass_rust.InstNoOp(
                            name=f"WSPLIT-{_num_splits[0]}", ins=[], outs=[]
                        )
                        nop.engine = inst.engine
                        nop.sync_info = mybir.SyncInfo(on_wait=[w], on_update=[])
                        new_list.append(nop)
                    changed = True
                new_list.append(inst)
            if changed:
                bb.instructions = new_list


# ---------------------------------------------------------------------------
# Device program (identical for all cores; per-core data via in_maps)
# ---------------------------------------------------------------------------
def build_program():
    _apply_patches()
    nc = bass.Bass()
    f32 = mybir.dt.float32
    i32 = mybir.dt.int32

    wv = nc.declare_dram_parameter("wv", [100352, D], f32, isOutput=False)
    toks = nc.declare_dram_parameter("toks", [P, F], i32, isOutput=False)
    segs = nc.declare_dram_parameter("segs", [P, F], i32, isOutput=False)
    hwT = nc.declare_dram_parameter("hwT", [D, NL], f32, isOutput=False)
    hb = nc.declare_dram_parameter("hb", [1, NL], f32, isOutput=False)
    out = nc.declare_dram_parameter("out", [SENT_PER_CORE, NL], f32, isOutput=True)

    bags = [nc.dram_tensor(f"bag{r}", [BAGROWS, 2], f32) for r in range(NBAG)]

    with ExitStack() as ctx:
        tc = ctx.enter_context(tile.TileContext(nc))
        const = ctx.enter_context(tc.tile_pool(name="const", bufs=1))
        gp = ctx.enter_context(tc.tile_pool(name="g", bufs=6))
        small = ctx.enter_context(tc.tile_pool(name="small", bufs=2))
        psum = ctx.enter_context(tc.tile_pool(name="ps", bufs=2, space="PSUM"))

        tok_sb = const.tile([P, F], i32)
        seg_sb = const.tile([P, F], i32)
        nc.sync.dma_start(tok_sb[:], toks[:])
        nc.sync.dma_start(seg_sb[:], segs[:])

        # zero-init the accumulators
        z = const.tile([P, 2 * BAGROWS // P], f32)
        nc.vector.memset(z[:], 0.0)
        for r in range(NBAG):
            nc.sync.dma_start(bags[r][:], z[:])

        # main loop: gather rows -> reduce -> scatter-add [emb, 1] into bag.
        # Payload tiles rotate through a pool so the per-column chains pipeline
        # (a single shared payload array would serialize every chain through
        # tile-granular dependency tracking).
        sp = ctx.enter_context(tc.tile_pool(name="s2", bufs=12))
        for k in range(F):
            g = gp.tile([P, D], f32, tag="g")
            nc.gpsimd.indirect_dma_start(
                out=g[:],
                out_offset=None,
                in_=wv[:],
                in_offset=IndirectOffsetOnAxis(ap=tok_sb[:, k : k + 1], axis=0),
            )
            s2 = sp.tile([P, 2], f32, tag="s2")
            nc.vector.memset(s2[:, 1:2], 1.0)
            nc.vector.tensor_reduce(
                s2[:, 0:1],
                g[:],
                axis=mybir.AxisListType.X,
                op=mybir.AluOpType.add,
            )
            nc.gpsimd.indirect_dma_start(
                out=bags[k % NBAG][:],
                out_offset=IndirectOffsetOnAxis(ap=seg_sb[:, k : k + 1], axis=0),
                in_=s2[:],
                in_offset=None,
                compute_op=mybir.AluOpType.add,
            )

        # combine the 128 accumulators: acc[p, u, c] = sum_r bag_r[u*128+p, c]
        acc = const.tile([P, UROWS, 2], f32)
        nc.vector.memset(acc[:], 0.0)
        for r in range(NBAG):
            w = gp.tile([P, UROWS, 2], f32, tag="w")
            nc.sync.dma_start(
                w[:],
                bags[r][:].rearrange("(u p) c -> p u c", p=P)[0:P, 0:UROWS, :],
            )
            nc.vector.tensor_tensor(
                out=acc[:], in0=acc[:], in1=w[:], op=mybir.AluOpType.add
            )

        # sv = seg_sum / max(len, 1)
        lens = small.tile([P, UROWS], f32)
        nc.vector.tensor_scalar_max(lens[:], acc[:, :, 1], 1.0)
        rec = small.tile([P, UROWS], f32)
        nc.vector.reciprocal(rec[:], lens[:])
        sv = small.tile([P, UROWS], f32)
        nc.vector.tensor_tensor(
            out=sv[:], in0=acc[:, :, 0], in1=rec[:], op=mybir.AluOpType.mult
        )

        # broadcast row-sums of hidden_w and the bias to all partitions
        hwT_sb = const.tile([D, NL], f32)
        nc.sync.dma_start(hwT_sb[:], hwT[:])
        hb_sb = const.tile([1, NL], f32)
        nc.sync.dma_start(hb_sb[:], hb[:])
        ones_p = const.tile([P, 1], f32)
        nc.vector.memset(ones_p[:], 1.0)
        ones_1 = const.tile([1, P], f32)
        nc.vector.memset(ones_1[:], 1.0)

        wrow_ps = psum.tile([1, NL], f32, tag="ps1")
        nc.tensor.matmul(wrow_ps[:], ones_p[:], hwT_sb[:], start=True, stop=True)
        wrow = small.tile([1, NL], f32)
        nc.scalar.copy(wrow[:], wrow_ps[:])

        wb_ps = psum.tile([P, NL], f32, tag="ps2")
        nc.tensor.matmul(wb_ps[:], ones_1[:], wrow[:], start=True, stop=True)
        w_b = const.tile([P, NL], f32)
        nc.scalar.copy(w_b[:], wb_ps[:])

        bb_ps = psum.tile([P, NL], f32, tag="ps2")
        nc.tensor.matmul(bb_ps[:], ones_1[:], hb_sb[:], start=True, stop=True)
        b_b = const.tile([P, NL], f32)
        nc.scalar.copy(b_b[:], bb_ps[:])

        # out[u*128+p, l] = sv[p, u] * w_b[p, l] + b_b[p, l]
        out_sb = const.tile([P, UROWS, NL], f32)
        for u in range(UROWS):
            nc.vector.tensor_scalar(
                out=out_sb[:, u, :],
                in0=w_b[:],
                scalar1=sv[:, u : u + 1],
                scalar2=None,
                op0=mybir.AluOpType.mult,
            )
            nc.vector.tensor_tensor(
                out=out_sb[:, u, :],
                in0=out_sb[:, u, :],
                in1=b_b[:],
                op=mybir.AluOpType.add,
            )

        nc.sync.dma_start(
            out[:].rearrange("(u p) l -> p u l", p=P)[0:P, 0:UROWS, :], out_sb[:]
        )

    _split_waits(nc)
    return nc


_PROGRAM = None


def _get_program():
    global _PROGRAM
    if _PROGRAM is None:
        _PROGRAM = build_program()
    return _PROGRAM


def kernel(tokens, segment_ids, word_vectors, hidden_w, hidden_b):
    from concourse.bass_utils import run_bass_kernel_spmd

    tokens = np.asarray(tokens)
    segment_ids = np.asarray(segment_ids)
    word_vectors = np.asarray(word_vectors, dtype=np.float32)
    hidden_w = np.asarray(hidden_w, dtype=np.float32)
    hidden_b = np.asarray(hidden_b, dtype=np.float32)

    # replicate-pad the embedding table to the declared 100352 rows
    wv_pad = np.zeros((100352, D), dtype=np.float32)
    wv_pad[: word_vectors.shape[0]] = word_vectors
    hwT = np.ascontiguousarray(hidden_w.T)
    hb = hidden_b.reshape(1, NL)

    # sentence-aligned cuts: core c owns sentences [2048c, 2048(c+1))
    cuts = np.searchsorted(segment_ids, np.arange(NCORES + 1) * SENT_PER_CORE)
    in_maps = []
    for c in range(NCORES):
        lo, hi = int(cuts[c]), int(cuts[c + 1])
        n = hi - lo
        assert n <= SHARD, f"shard {c} has {n} tokens > {SHARD}"
        tk = np.zeros(SHARD, dtype=np.int32)
        sg = np.full(SHARD, PAD_SEG, dtype=np.int32)
        tk[:n] = tokens[lo:hi]
        sg[:n] = segment_ids[lo:hi] - c * SENT_PER_CORE
        in_maps.append(
            {
                "wv": wv_pad,
                "toks": tk.reshape(P, F),
                "segs": sg.reshape(P, F),
                "hwT": hwT,
                "hb": hb,
            }
        )

    nc = _get_program()
    res = run_bass_kernel_spmd(nc, in_maps, list(range(NCORES)))
    return np.concatenate([res.results[c]["out"] for c in range(NCORES)], axis=0)
